# revision 1
# baseline (speedup 1.0000x reference)
"""Trainium2 Bass kernel for nn_CAD_GCN (gnn_message_passing).

Math: with x [B,C,H,W], S = H*W, x_node = mean_s x,
  h   = x_node @ g1_w.T + g1_b
  z1  = h*g2_w + g2_b
  y   = sum_n (theta_w x + theta_b)[n] * z1[n]
      = sum_c w_eff[c]*x[c,s] + bias_eff          (no Bmap materialization)
  out = tanh(x + phi_w[c]*y + phi_b[c])
where w_eff = x_node @ A + r, bias_eff = x_node @ a + s0 with
  A = g2_w*(g1_w.T @ theta_w), r = (g2_w*g1_b + g2_b) @ theta_w
  a = g2_w*(g1_w.T @ theta_b), s0 = (g2_w*g1_b + g2_b) @ theta_b
(all host-precomputable from the tiny parameter tensors).

Sharding: pure data parallel, 2 samples per core on 8 cores. Each core
sees its x slice as [128, 65536] (partition p = (b_local, c)).

Device program per core:
  pass 1: stream x chunks, free-dim reduce -> sums [128,1]
  tiny:   w2 [128,2] = mbd.T @ sums_bd + rbd;  M2 = w2 @ phi2 [128,128]
          (per-sample rank-1 map), bias2 [128,1] via abd
  pass 2: per 512-col tile: z = M2.T @ x (one PE matmul, f32r),
          s = x + z (DVE), out = tanh(s + bias2) (ACT), DMA out.
The first RETAIN chunks stay resident in SBUF between the passes to cut
HBM re-read traffic; the big matmuls run in float32r (fast fp32 PE mode,
~1e-4 relative error vs ~2e-2-style gates).
"""

import sys

for _p in ("/opt/trn_rl_repo",):
    if _p not in sys.path:
        sys.path.insert(0, _p)

import numpy as np

import concourse.bacc as bacc
import concourse.bass as bass
import concourse.mybir as mybir
import concourse.tile as tile
from concourse.bass_utils import run_bass_kernel_spmd

F32 = mybir.dt.float32
F32R = mybir.dt.float32r

B, C, H, W = 16, 64, 256, 256
S = H * W                      # 65536 pixels per sample
NCORES = 8
BPC = B // NCORES              # 2 samples per core
P = BPC * C                    # 128 partitions = (b_local, c)

CHUNK = 2048                   # free-dim columns per DMA (1 MiB per chunk)
SUB = 512                      # matmul free-dim tile (one fp32 PSUM bank)
RETAIN = 18                    # chunks kept in SBUF between pass 1 and 2
USE_F32R = True                # fast fp32 matmul mode for the big matmuls

NCHUNK = S // CHUNK
NSUB = CHUNK // SUB
INV_S = 1.0 / float(S)


def _build_program(n_pix=S, chunk=CHUNK, retain=RETAIN, use_f32r=USE_F32R,
                   xstream_bufs=3, spool_bufs=2, opool_bufs=4,
                   psy_bufs=3, psz_bufs=3, store_eng="sync", load2_eng="gpsimd",
                   lc=None, sc=None, ypool_bufs=4, inplace=True, zwide=1, order_mode=0,
                   rl=2, load1_eng="sync", retload_eng="sync",
                   act_reduce=True):
    """chunk: compute granularity (DVE/ACT/PE tiling, and spool size).
    lc: load-chunk (DMA granularity for x, multiple of chunk). sc: store-chunk.
    retain counts load-chunks."""
    lc = lc or chunk
    sc = sc or chunk
    assert lc % chunk == 0 and sc % chunk == 0 and n_pix % lc == 0
    nload = n_pix // lc
    cpl = lc // chunk              # compute chunks per load chunk
    cps = sc // chunk              # compute chunks per store chunk
    nsub = chunk // SUB if chunk >= SUB else 1
    sub = min(SUB, chunk)

    nc = bacc.Bacc("TRN2", target_bir_lowering=False, debug=False)

    # f32r is bit-identical to f32; the PE's fast fp32 matmul path requires
    # its operands to be *typed* f32r end-to-end. DVE/ACT consumers bitcast
    # back to plain f32.
    XD = F32R if use_f32r else F32

    x_d = nc.dram_tensor("x", [P, n_pix], XD, kind="ExternalInput")
    mbd_d = nc.dram_tensor("mbd", [P, P], F32, kind="ExternalInput")
    abd_d = nc.dram_tensor("abd", [P, P], F32, kind="ExternalInput")
    rbd_d = nc.dram_tensor("rbd", [P, 2], F32, kind="ExternalInput")
    bcol_d = nc.dram_tensor("bcol", [P, 1], F32, kind="ExternalInput")
    phi2_d = nc.dram_tensor("phi2", [2, P], F32, kind="ExternalInput")
    eye_d = nc.dram_tensor("eye", [P, P], F32, kind="ExternalInput")
    out_d = nc.dram_tensor("out", [P, n_pix], F32, kind="ExternalOutput")

    X = mybir.AxisListType.X
    Tanh = mybir.ActivationFunctionType.Tanh

    def asf32(ap):
        return ap.bitcast(F32) if use_f32r else ap

    with tile.TileContext(nc) as tc:
        with (
            tc.tile_pool(name="consts", bufs=1) as consts,
            tc.tile_pool(name="xstream", bufs=xstream_bufs) as xpool,
            tc.tile_pool(name="xret", bufs=1) as rpool,
            tc.tile_pool(name="stats", bufs=1) as stats,
            tc.tile_pool(name="spool", bufs=spool_bufs) as spool,
            tc.tile_pool(name="opool", bufs=opool_bufs) as opool,
            tc.tile_pool(name="ps_small", bufs=1, space="PSUM") as ps_small,
            tc.tile_pool(name="ps_z", bufs=psz_bufs, space="PSUM") as ps_z,
        ):
            # ---- constants to SBUF ----
            mbd_sb = consts.tile([P, P], F32, name="mbd_sb")
            nc.gpsimd.dma_start(mbd_sb[:], mbd_d[:])
            abd_sb = consts.tile([P, P], F32, name="abd_sb")
            nc.gpsimd.dma_start(abd_sb[:], abd_d[:])
            rbd_sb = consts.tile([P, 2], F32, name="rbd_sb")
            nc.gpsimd.dma_start(rbd_sb[:], rbd_d[:])
            bcol_sb = consts.tile([P, 1], F32, name="bcol_sb")
            nc.gpsimd.dma_start(bcol_sb[:], bcol_d[:])
            phi2_sb = consts.tile([2, P], F32, name="phi2_sb")
            nc.gpsimd.dma_start(phi2_sb[:], phi2_d[:])
            eye_sb = consts.tile([P, P], F32, name="eye_sb")
            nc.gpsimd.dma_start(eye_sb[:], eye_d[:])

            # ---- pass 1: channel sums ----
            # Retained chunks live in one contiguous mega-tile, loaded with a
            # few multi-MB DMAs (HBM efficiency rises sharply with transfer
            # size); reduces still run per load-chunk on subtile slices.
            sums_nk = stats.tile([P, nload], F32, name="sums_nk")
            xret = rpool.tile([P, retain * lc], XD, name="xret") if retain else None
            for g0 in range(0, retain, rl):
                g1 = min(g0 + rl, retain)
                getattr(nc, retload_eng).dma_start(
                    xret[:, g0 * lc : g1 * lc], x_d[:, g0 * lc : g1 * lc]
                )
            Copy = mybir.ActivationFunctionType.Copy
            xtiles = []
            for i in range(nload):
                if i < retain:
                    xt = xret[:, i * lc : (i + 1) * lc]
                else:
                    xt = xpool.tile([P, lc], XD, name="xs", tag="xs")
                    getattr(nc, load1_eng).dma_start(
                        xt[:], x_d[:, i * lc : (i + 1) * lc]
                    )
                if act_reduce and i >= retain and i % 2 == 1:
                    # balance pass-1 reductions across DVE and ACT: the
                    # activation computes the free-dim sum via accum_out; the
                    # copy output is written in place (streamed tiles have no
                    # other consumers in pass 1, so this costs no SBUF)
                    nc.scalar.activation(
                        asf32(xt[:]), asf32(xt[:]), Copy,
                        accum_out=sums_nk[:, i : i + 1],
                    )
                else:
                    nc.vector.reduce_sum(sums_nk[:, i : i + 1], asf32(xt[:]), X)
                xtiles.append(xt if i < retain else None)

            sums = stats.tile([P, 1], F32, name="sums")
            nc.vector.reduce_sum(sums[:, 0:1], sums_nk[:], X)

            # block-diagonal copy of sums: col j holds sample j's sums
            sums_bd = stats.tile([P, 2], F32, name="sums_bd")
            nc.vector.memset(sums_bd[:], 0.0)
            nc.vector.tensor_copy(sums_bd[0:C, 0:1], sums[0:C, 0:1])
            nc.vector.tensor_copy(sums_bd[C:P, 1:2], sums[C:P, 0:1])

            # ---- tiny stage: w2 [P,2] and bias2 [P,1] ----
            w2_ps = ps_small.tile([P, 2], F32, name="w2_ps", tag="tiny")
            nc.tensor.matmul(w2_ps[:], mbd_sb[:], sums_bd[:], start=True, stop=True)
            w2_sb = stats.tile([P, 2], F32, name="w2_sb")
            nc.vector.tensor_add(w2_sb[:], w2_ps[:], rbd_sb[:])

            # M2 = w2 @ phi2  [P, P]: per-sample rank-1 map so that
            # z = M2.T @ x directly (one matmul per tile in pass 2)
            w2T_ps = ps_small.tile([2, P], F32, name="w2T_ps", tag="tiny2")
            nc.tensor.transpose(w2T_ps[:], w2_sb[:], eye_sb[:])
            w2T_sb = stats.tile([2, P], F32, name="w2T_sb")
            nc.scalar.copy(w2T_sb[:], w2T_ps[:])
            M2_ps = ps_small.tile([P, P], F32, name="M2_ps", tag="tiny3")
            nc.tensor.matmul(M2_ps[:], w2T_sb[:], phi2_sb[:], start=True, stop=True)
            M2_r = stats.tile([P, P], XD, name="M2_r")
            nc.vector.tensor_copy(M2_r[:], M2_ps[:])

            b2_ps = ps_small.tile([P, 2], F32, name="b2_ps", tag="tiny")
            nc.tensor.matmul(b2_ps[:], abd_sb[:], sums_bd[:], start=True, stop=True)
            b2_tmp = stats.tile([P, 1], F32, name="b2_tmp")
            nc.vector.reduce_sum(b2_tmp[:, 0:1], b2_ps[:], X)
            bias2 = stats.tile([P, 1], F32, name="bias2")
            nc.vector.tensor_add(bias2[:], b2_tmp[:], bcol_sb[:])

            # ---- pass 2 ----
            ncomp = n_pix // chunk
            ot = None
            # streamed chunks first: their loads overlap the tiny stage, and
            # the kernel tail lands on SBUF-resident retained chunks.
            # order_mode mixes some retained chunks into the streamed phase to
            # smooth the transition.
            streamed = [i for i in range(ncomp) if i // cpl >= retain]
            retained = [i for i in range(ncomp) if i // cpl < retain]
            if order_mode == 0:
                order = streamed + retained
            else:
                order = []
                a = b = 0
                while a < len(streamed) or b < len(retained):
                    for _ in range(order_mode):
                        if a < len(streamed):
                            order.append(streamed[a]); a += 1
                    if b < len(retained):
                        order.append(retained[b]); b += 1
            for i in order:
                li, lj = divmod(i, cpl)            # load-chunk index / offset
                if lj == 0:
                    if li < retain:
                        xt = xtiles[li]
                    else:
                        xt = xpool.tile([P, lc], XD, name="xs", tag="xs")
                        getattr(nc, load2_eng).dma_start(
                            xt[:], x_d[:, li * lc : (li + 1) * lc]
                        )
                si, sj = divmod(i, cps)
                if sj == 0:
                    ot = opool.tile([P, sc], F32, name="ot", tag="ot")
                if inplace:
                    st = ot
                    soff = sj * chunk
                else:
                    st = spool.tile([P, chunk], F32, name="st", tag="st")
                    soff = 0
                # zwide: one PSUM tile spanning `zwide` banks; matmuls fill
                # 512-wide bank-aligned slices, one DVE add covers them all
                zw = sub * zwide
                for jz in range(max(1, chunk // zw)):
                    z_ps = ps_z.tile([P, zw], F32, name="z_ps", tag="z")
                    for j in range(zwide):
                        off = jz * zw + j * sub
                        gsl = slice(lj * chunk + off, lj * chunk + off + sub)
                        nc.tensor.matmul(
                            z_ps[:, j * sub : (j + 1) * sub], M2_r[:], xt[:, gsl],
                            start=True, stop=True,
                        )
                    gz = slice(lj * chunk + jz * zw, lj * chunk + (jz + 1) * zw)
                    nc.vector.tensor_add(
                        st[:, soff + jz * zw : soff + (jz + 1) * zw],
                        asf32(xt[:, gz]), z_ps[:],
                    )
                nc.scalar.activation(
                    ot[:, sj * chunk : (sj + 1) * chunk],
                    st[:, soff : soff + chunk], Tanh,
                    bias=bias2[:, 0:1],
                )
                if sj == cps - 1:
                    getattr(nc, store_eng).dma_start(
                        out_d[:, si * sc : (si + 1) * sc], ot[:]
                    )

    nc.compile()
    return nc


def _host_consts(theta_w, theta_b, g1_w, g1_b, g2_w, g2_b, phi_w, phi_b):
    """Fold the GCN parameter chain into the device-side constant tensors."""
    f8 = np.float64
    theta_w = theta_w.astype(f8)
    theta_b = theta_b.astype(f8)
    g1_w = g1_w.astype(f8)
    g1_b = g1_b.astype(f8)
    g2w = f8(g2_w.reshape(-1)[0])
    g2b = f8(g2_b.reshape(-1)[0])
    phi_w = phi_w.astype(f8)
    phi_b = phi_b.astype(f8)

    # w_eff = x_node @ A + r ; bias_eff = x_node @ a + s0
    A = g2w * (g1_w.T @ theta_w)            # [C, C]
    r = (g2w * g1_b + g2b) @ theta_w        # [C]
    a = g2w * (g1_w.T @ theta_b)            # [C]
    s0 = (g2w * g1_b + g2b) @ theta_b       # scalar

    # mbd[p', p] = ind(b(p')==b(p)) * A[c(p'), c(p)] / S
    mbd = np.zeros((P, P), f8)
    mbd[0:C, 0:C] = A * INV_S
    mbd[C:P, C:P] = A * INV_S
    # abd[p', p] = ind(b(p')==b(p)) * phi_w[c(p)] * a[c(p')] / S
    abd = np.zeros((P, P), f8)
    abd[0:C, 0:C] = np.outer(a, phi_w) * INV_S
    abd[C:P, C:P] = np.outer(a, phi_w) * INV_S
    # rbd[p, j] = ind(b(p)==j) * r[c(p)]
    rbd = np.zeros((P, 2), f8)
    rbd[0:C, 0] = r
    rbd[C:P, 1] = r
    # bcol[p] = phi_w[c]*s0 + phi_b[c]
    bcol = np.tile(phi_w * s0 + phi_b, BPC)[:, None]
    # phi2[j, p] = ind(b(p)==j) * phi_w[c(p)]
    phi2 = np.zeros((2, P), f8)
    phi2[0, 0:C] = phi_w
    phi2[1, C:P] = phi_w

    c32 = lambda t: np.ascontiguousarray(t, dtype=np.float32)
    return {
        "mbd": c32(mbd),
        "abd": c32(abd),
        "rbd": c32(rbd),
        "bcol": c32(bcol),
        "phi2": c32(phi2),
        "eye": c32(np.eye(P)),
    }


_NC_CACHE = {}


def _get_nc():
    key = (S, CHUNK, RETAIN, USE_F32R)
    if key not in _NC_CACHE:
        _NC_CACHE[key] = _build_program(S, CHUNK, RETAIN, USE_F32R)
    return _NC_CACHE[key]


def _run(inputs, trace=False):
    x = np.ascontiguousarray(np.asarray(inputs["x"]), dtype=np.float32)
    consts = _host_consts(
        np.asarray(inputs["theta_w"]), np.asarray(inputs["theta_b"]),
        np.asarray(inputs["g1_w"]), np.asarray(inputs["g1_b"]),
        np.asarray(inputs["g2_w"]), np.asarray(inputs["g2_b"]),
        np.asarray(inputs["phi_w"]), np.asarray(inputs["phi_b"]),
    )
    in_maps = []
    for k in range(NCORES):
        xk = x[k * BPC : (k + 1) * BPC].reshape(P, S)
        in_maps.append({"x": np.ascontiguousarray(xk), **consts})

    nc = _get_nc()
    res = run_bass_kernel_spmd(
        nc, in_maps, core_ids=list(range(NCORES)), trace=trace
    )
    out = np.empty((B, C, H, W), dtype=np.float32)
    for k in range(NCORES):
        out[k * BPC : (k + 1) * BPC] = res.results[k]["out"].reshape(BPC, C, H, W)
    return out, res


def kernel(**inputs):
    out, _ = _run(inputs, trace=False)
    return out



# revision 8
# speedup vs baseline: 2.3886x; 2.3886x over previous
"""Trainium2 Bass kernel for nn_CAD_GCN (gnn_message_passing).

Math: with x [B,C,H,W], S = H*W, x_node = mean_s x,
  h   = x_node @ g1_w.T + g1_b
  z1  = h*g2_w + g2_b
  y   = sum_n (theta_w x + theta_b)[n] * z1[n]
      = sum_c w_eff[c]*x[c,s] + bias_eff          (no Bmap materialization)
  out = tanh(x + phi_w[c]*y + phi_b[c])
where w_eff = x_node @ A + r, bias_eff = x_node @ a + s0 with
  A = g2_w*(g1_w.T @ theta_w), r = (g2_w*g1_b + g2_b) @ theta_w
  a = g2_w*(g1_w.T @ theta_b), s0 = (g2_w*g1_b + g2_b) @ theta_b
(all host-precomputable from the tiny parameter tensors).

Sharding: pure data parallel, 2 samples per core on 8 cores.

The error gate (2e-2 absmax-relative) admits bf16 at the HBM boundary:
x is uploaded as bf16 and the output is downloaded as bf16 (host casts
either way; worst-case output perturbation ~5e-3).  That halves DMA
traffic AND lets loads land directly in the retained SBUF image of x —
no staging, no convert pass.  Per core, per sample (viewed as
[128, 32768] bf16, partition p = (c, half)):

  loads:  16x 2048-col DMAs straight into the resident xret tile.
  sums:   DVE in-place tensor_scalar(mult 1.0) per chunk with fused
          accum_out — runs in the DVE 16-bit fast mode (689ns/2048),
          comfortably ahead of the 1456ns/chunk DMA stream.
  tiny:   w_eff/bias via two small PE matmuls + DVE;
          M2b = I + parity*(w_eff outer phi) in bf16, so pass 2 is
          z = M2b.T @ x = x + phi*y directly (no elementwise add).
  pass 2: per 2048-col PSUM tile (4 banks, 2-deep ring): 4 bf16 PE
          matmuls + ONE 2048-wide ACT tanh(bias) into bf16 staging,
          stored per 4096 cols from gpsimd.

ACT (tanh, 32x 1882ns = 60us) is the critical engine; DMA busy is only
~70us.  Sample A's load phase (23us) is the serial head; sample B's
loads/sums/tiny all hide under A's tanh stream.
"""

import sys

for _p in ("/opt/trn_rl_repo",):
    if _p not in sys.path:
        sys.path.insert(0, _p)

import numpy as np

import concourse.bacc as bacc
import concourse.bass as bass
import concourse.mybir as mybir
import concourse.tile as tile
from concourse.bass_utils import run_bass_kernel_spmd

F32 = mybir.dt.float32
BF16 = mybir.dt.bfloat16
NP_BF16 = mybir.dt.np(BF16)

B, C, H, W = 16, 64, 256, 256
S = H * W                      # 65536 pixels per sample
NCORES = 8
BPC = B // NCORES              # 2 samples per core
P = 128                        # SBUF partitions; per sample p = 2*c + half
SPS = S // 2                   # 32768 pixels per virtual half-sample column
INV_S = 1.0 / float(S)

LC = 2048                      # load chunk (bf16 cols)
SC = 2048                      # store chunk (bf16 cols)
ZW = 2048                      # PSUM z tile width (4 banks)
NLOAD = SPS // LC              # 16 loads per sample
NZ = SPS // ZW                 # 16 z-chunks per sample
ZPS = SC // ZW                 # z-chunks per store (2)


def _build_program(ot_bufs=10, psz_bufs=2, load_eng="sync", store_eng="gpsimd"):
    nc = bacc.Bacc("TRN2", target_bir_lowering=False, debug=False)

    x_d = nc.dram_tensor("x", [2 * P, SPS], BF16, kind="ExternalInput")
    consts_d = nc.dram_tensor("consts", [P, 516], F32, kind="ExternalInput")
    out_d = nc.dram_tensor("out", [2 * P, SPS], BF16, kind="ExternalOutput")

    X = mybir.AxisListType.X
    Tanh = mybir.ActivationFunctionType.Tanh
    Mult = mybir.AluOpType.mult
    Add = mybir.AluOpType.add

    with tile.TileContext(nc) as tc:
        with (
            tc.tile_pool(name="consts", bufs=1) as cpool,
            tc.tile_pool(name="xret", bufs=1) as rpool,
            tc.tile_pool(name="stats", bufs=1) as stats,
            tc.tile_pool(name="opool", bufs=ot_bufs) as opool,
            tc.tile_pool(name="ps_z", bufs=psz_bufs, space="PSUM") as ps_z,
        ):
            consts_sb = cpool.tile([P, 516], F32, name="consts_sb")
            getattr(nc, load_eng).dma_start(consts_sb[:], consts_d[:])
            mbd_sb = consts_sb[:, 0:128]
            abd_sb = consts_sb[:, 128:256]
            pphi_sb = consts_sb[:, 256:384]
            eye_sb = consts_sb[:, 384:512]
            rcol_sb = consts_sb[:, 512:513]
            bcol_sb = consts_sb[:, 513:514]

            xret = [rpool.tile([P, SPS], BF16, name=f"xret{s}") for s in range(2)]
            snk = [stats.tile([P, NLOAD], F32, name=f"snk{s}") for s in range(2)]

            def emit_load_sum(s, i):
                sl = slice(i * LC, (i + 1) * LC)
                getattr(nc, load_eng).dma_start(
                    xret[s][:, sl], x_d[s * P : (s + 1) * P, sl]
                )
                # in-place mult-by-1 (bitwise identity on bf16) purely to
                # ride the DVE 16-bit fast path while harvesting the sum
                with nc.allow_low_precision(reason="bf16 identity copy; sum accumulates in f32"):
                    nc.vector.tensor_scalar(
                        xret[s][:, sl], xret[s][:, sl], 1.0, 0.0, Mult, Add,
                        accum_out=snk[s][:, i : i + 1],
                    )

            M2b = [None, None]
            bias2 = [None, None]

            def emit_tiny(s):
                sums = stats.tile([P, 1], F32, name=f"sums{s}")
                nc.vector.reduce_sum(sums[:, 0:1], snk[s][:], X)
                w2raw = ps_z.tile([P, ZW], F32, name="w2raw", tag="z")
                nc.tensor.matmul(w2raw[:, 0:1], mbd_sb, sums[:, 0:1], start=True, stop=True)
                b2raw = ps_z.tile([P, ZW], F32, name="b2raw", tag="z")
                nc.tensor.matmul(b2raw[:, 0:1], abd_sb, sums[:, 0:1], start=True, stop=True)
                w2col = stats.tile([P, 1], F32, name=f"w2col{s}")
                nc.vector.tensor_add(w2col[:], w2raw[:, 0:1], rcol_sb)
                b2 = stats.tile([P, 1], F32, name=f"bias2_{s}")
                nc.vector.tensor_add(b2[:], b2raw[:, 0:1], bcol_sb)
                m2t = stats.tile([P, P], F32, name=f"m2t{s}")
                nc.vector.tensor_scalar_mul(m2t[:], pphi_sb, w2col[:, 0:1])
                m2b = stats.tile([P, P], BF16, name=f"m2b{s}")
                nc.vector.tensor_add(m2b[:], m2t[:], eye_sb)
                M2b[s] = m2b
                bias2[s] = b2

            ot_cur = [None, None]

            def emit_z_chunk(s, zi):
                if zi % ZPS == 0:
                    ot_cur[s] = opool.tile([P, SC], BF16, name="ot", tag="ot")
                ot = ot_cur[s]
                z_ps = ps_z.tile([P, ZW], F32, name="z", tag="z")
                for j in range(ZW // 512):
                    g0 = zi * ZW + j * 512
                    nc.tensor.matmul(
                        z_ps[:, j * 512 : (j + 1) * 512], M2b[s][:],
                        xret[s][:, g0 : g0 + 512], start=True, stop=True,
                    )
                off = (zi % ZPS) * ZW
                nc.scalar.activation(
                    ot[:, off : off + ZW], z_ps[:], Tanh, bias=bias2[s][:, 0:1]
                )
                if zi % ZPS == ZPS - 1:
                    si = zi // ZPS
                    getattr(nc, store_eng).dma_start(
                        out_d[s * P : (s + 1) * P, si * SC : (si + 1) * SC], ot[:]
                    )

            # phase 1: sample A in
            for i in range(NLOAD):
                emit_load_sum(0, i)
            emit_tiny(0)
            # phase 2: sample B in, 1:1 with A pass 2
            za = 0
            for i in range(NLOAD):
                emit_load_sum(1, i)
                if za < NZ:
                    emit_z_chunk(0, za)
                    za += 1
            emit_tiny(1)
            # phase 3: finish A (nothing left when NLOAD==NZ), then B
            while za < NZ:
                emit_z_chunk(0, za)
                za += 1
            for zb in range(NZ):
                emit_z_chunk(1, zb)

    nc.compile()
    return nc


def _host_consts(theta_w, theta_b, g1_w, g1_b, g2_w, g2_b, phi_w, phi_b):
    """Fold the GCN parameter chain into one packed [128, 516] tensor."""
    f8 = np.float64
    theta_w = theta_w.astype(f8)
    theta_b = theta_b.astype(f8)
    g1_w = g1_w.astype(f8)
    g1_b = g1_b.astype(f8)
    g2w = f8(g2_w.reshape(-1)[0])
    g2b = f8(g2_b.reshape(-1)[0])
    phi_w = phi_w.astype(f8)
    phi_b = phi_b.astype(f8)

    A = g2w * (g1_w.T @ theta_w)            # [C, C]
    r = (g2w * g1_b + g2b) @ theta_w        # [C]
    a = g2w * (g1_w.T @ theta_b)            # [C]
    s0 = (g2w * g1_b + g2b) @ theta_b       # scalar

    rep = lambda v: np.repeat(v, 2)         # c = p // 2
    # w2col[p'] = sum_p mbd[p,p'] * sums[p] (+ rcol) = w_eff[c(p')]
    mbd = np.repeat(np.repeat(A, 2, axis=0), 2, axis=1) * INV_S
    # b2[p'] = sum_p abd[p,p'] * sums[p] (+ bcol) = phi_w[c(p')]*s_b + ...
    abd = np.outer(rep(a), rep(phi_w)) * INV_S
    # pphi[p,p'] = (p%2 == p'%2) * phi_w[c(p')]
    par = (np.arange(P)[:, None] % 2) == (np.arange(P)[None, :] % 2)
    pphi = par * rep(phi_w)[None, :]
    rcol = rep(r)
    bcol = rep(phi_w * s0 + phi_b)

    consts = np.zeros((P, 516), f8)
    consts[:, 0:128] = mbd
    consts[:, 128:256] = abd
    consts[:, 256:384] = pphi
    consts[:, 384:512] = np.eye(P)
    consts[:, 512] = rcol
    consts[:, 513] = bcol
    return np.ascontiguousarray(consts, dtype=np.float32)


_NC_CACHE = {}


def _get_nc():
    key = (S, LC, SC, ZW)
    if key not in _NC_CACHE:
        _NC_CACHE[key] = _build_program()
    return _NC_CACHE[key]


def _run(inputs, trace=False):
    x = np.asarray(inputs["x"]).astype(NP_BF16)
    consts = _host_consts(
        np.asarray(inputs["theta_w"]), np.asarray(inputs["theta_b"]),
        np.asarray(inputs["g1_w"]), np.asarray(inputs["g1_b"]),
        np.asarray(inputs["g2_w"]), np.asarray(inputs["g2_b"]),
        np.asarray(inputs["phi_w"]), np.asarray(inputs["phi_b"]),
    )
    in_maps = []
    for k in range(NCORES):
        xk = x[k * BPC : (k + 1) * BPC].reshape(2 * P, SPS)
        in_maps.append({"x": np.ascontiguousarray(xk), "consts": consts})

    nc = _get_nc()
    res = run_bass_kernel_spmd(
        nc, in_maps, core_ids=list(range(NCORES)), trace=trace
    )
    out = np.empty((B, C, H, W), dtype=np.float32)
    for k in range(NCORES):
        out[k * BPC : (k + 1) * BPC] = (
            np.asarray(res.results[k]["out"])
            .astype(np.float32)
            .reshape(BPC, C, H, W)
        )
    return out, res


def kernel(**inputs):
    out, _ = _run(inputs, trace=False)
    return out
